# revision 31
# baseline (speedup 1.0000x reference)
"""GPS layer (GCN + dense Performer attention + FFN) on 8 Trainium2 cores.

Strategy (per core, rows R=1024 of N=8192 nodes):
  - GCN segment-sum as a dense matmul with the normalized adjacency
    A = D^-1/2 (Adj + I) D^-1/2, built host-side, shipped fp8-e4m3 in an
    lhsT-friendly layout, computed as (A @ h) @ W_gcn with h also e4m3 so
    the big matmul runs in DoubleRow mode (2 moving pixels/cycle).
  - Attention in transposed score layout ST[c, r] = kf@qf^T: softmax via
    the appended ones-column of V (no max subtraction needed in bf16),
    exp(ST) tiles are directly the lhsT of the P@V matmul.  qf is bf16
    (local only), kf/v fp8-e3m4 on the wire.
  - k-features and V are all-gathered in QUARTER payloads (132KB/rank,
    4 collectives) fired as soon as each pair of GCN row blocks clears,
    so the gather pipeline runs behind the remaining GCN work and the
    first payload lands before phase 2 starts.  kfa/vaug live in
    per-quarter SBUF tiles so a score chunk only depends on its own
    quarter's unpack (tile-granular dependency tracking).  Pack writes
    ride the scalar queue; kfa unpacks the sync ring, vaug the gpsimd
    ring -- an unpack can never sit behind a later quarter's pack.
  - Phase 2 in arrival order at quarter granularity: scores+exp per
    quarter, half-A P@V partial saved to SBUF, half-B P@V + output
    projection + FFN per slab.
"""

import os
import sys

sys.path.insert(0, "/opt/trn_rl_repo")
os.environ.setdefault("MYCRO_LOCAL_CACHE", "1")

import numpy as np
import ml_dtypes

import concourse.bass as bass
import concourse.tile as tile
from concourse import bacc, mybir
from concourse.bass_utils import run_bass_kernel_spmd
from concourse.masks import make_identity

f32 = mybir.dt.float32
bf16 = mybir.dt.bfloat16
f8e3 = mybir.dt.float8e3
f8e4 = mybir.dt.float8e4
DR = mybir.MatmulPerfMode.DoubleRow
BF = ml_dtypes.bfloat16
E3 = ml_dtypes.float8_e3m4
E4 = ml_dtypes.float8_e4m3

N, D, F, M = 8192, 256, 512, 256
NCORES = 8
R = N // NCORES          # rows per core (1024)
RB = R // 128            # row blocks per core (8)
KC = D // 128            # feature chunks (2)
NCH = N // 128           # node chunks (64)
FC = F // 128            # ffn chunks (4)
VA = 260                 # v free dim: 256 features + ones col + pad
EPS = 1e-5
NQ = 4                   # collective quarters
RQ = R // NQ             # rows per quarter (256)
KFTQ_B = M * RQ          # kft bytes per quarter (e3m4): 65536
VAUGQ_B = RQ * VA        # vaug bytes per quarter (e3m4): 66560
PACKQ = KFTQ_B + VAUGQ_B # 132096 bytes per rank per quarter


def _ln_block(nc, pool, x_sb, out_sb, g_bc=None, be_bc=None, eps_t=None,
              tail=None):
    """LayerNorm over free dim (256) of x_sb [128, 256] f32 -> out_sb."""
    tail = tail or nc.vector
    stats = pool.tile([128, 6], f32, tag="ln_stats")
    nc.vector.bn_stats(stats[:], x_sb)
    mv = pool.tile([128, 2], f32, tag="ln_mv")
    nc.vector.bn_aggr(mv[:], stats[:])
    nc.scalar.activation(mv[:, 1:2], mv[:, 1:2],
                         mybir.ActivationFunctionType.Sqrt, bias=eps_t)
    nc.vector.reciprocal(mv[:, 1:2], mv[:, 1:2])
    nc.vector.tensor_scalar(out=out_sb, in0=x_sb,
                            scalar1=mv[:, 0:1], scalar2=mv[:, 1:2],
                            op0=mybir.AluOpType.subtract,
                            op1=mybir.AluOpType.mult)
    if g_bc is not None:
        tail.tensor_mul(out_sb, out_sb, g_bc)
        tail.tensor_add(out_sb, out_sb, be_bc)


def _build():
    nc = bacc.Bacc("TRN2", target_bir_lowering=False, debug=False,
                   num_devices=NCORES)

    def inp(name, shape, dt):
        return nc.dram_tensor(name, shape, dt, kind="ExternalInput")

    at_h = inp("at", [RB, 2, 128, NCH // 2, 128], f8e4)
    hn_h = inp("hn", [128, NCH * D], f8e4)
    hres_h = inp("hres", [128, RB * D], f32)
    wgcn_h = inp("wgcn", [D, D], bf16)
    wq_h = inp("wq", [D, D], bf16)
    wk_h = inp("wk", [D, D], bf16)
    wv_h = inp("wv", [D, D], bf16)
    wo_h = inp("wo", [D, D], bf16)
    rft_h = inp("rft", [D, M], bf16)
    w1_h = inp("w1", [D, F], bf16)
    w2_h = inp("w2", [F, D], bf16)
    bq_h = inp("bqc", [D, 1], f32)
    bk_h = inp("bkc", [D, 1], f32)
    b1c_h = inp("b1c", [F, 1], f32)
    bvr_h = inp("bvr", [1, D], bf16)
    bor_h = inp("bor", [1, D], bf16)
    b2r_h = inp("b2r", [1, D], bf16)
    gb_h = {}
    for nm in ("g1", "be1", "g2", "be2", "g3", "be3"):
        gb_h[nm] = inp(nm, [1, D], bf16)

    out_h = nc.dram_tensor("out", [R, D], f32, kind="ExternalOutput")

    with tile.TileContext(nc) as tc:
        _body(tc, at_h, hn_h, hres_h, wgcn_h, wq_h, wk_h, wv_h, wo_h, rft_h,
              w1_h, w2_h, bq_h, bk_h, b1c_h, bvr_h, bor_h, b2r_h, gb_h, out_h)
    nc.compile()
    return nc


def _body(tc, at_h, hn_h, hres_h, wgcn_h, wq_h, wk_h, wv_h, wo_h, rft_h,
          w1_h, w2_h, bq_h, bk_h, b1c_h, bvr_h, bor_h, b2r_h, gb_h, out_h):
    from contextlib import ExitStack
    nc = tc.nc
    Exp = mybir.ActivationFunctionType.Exp
    Copy = mybir.ActivationFunctionType.Copy
    Relu = mybir.ActivationFunctionType.Relu
    ADD = mybir.AluOpType.add

    with ExitStack() as octx:
        const = octx.enter_context(tc.tile_pool(name="const", bufs=1))
        persist = octx.enter_context(tc.tile_pool(name="persist", bufs=1))
        dram = octx.enter_context(tc.tile_pool(name="dram", bufs=1, space="DRAM"))

        # ---- persistent activations (per-quarter kfa/vaug tiles) ----
        kfaq = [persist.tile([128, KC, NCORES * RQ], f8e3, tag=f"kfaq{qt}",
                             name=f"kfaq{qt}") for qt in range(NQ)]
        vaugq = [persist.tile([128, NCORES * 2, VA], f8e3, tag=f"vaugq{qt}",
                              name=f"vaugq{qt}") for qt in range(NQ)]
        h1_sb = persist.tile([128, RB, D], f32, tag="h1")
        qft_sb = persist.tile([128, KC, R], bf16, tag="qft")

        # ---- packed collective DRAM buffers (four quarters) ----
        pack_loc = [dram.tile([PACKQ], f8e3, tag=f"pk_loc{qt}",
                              name=f"pk_loc{qt}") for qt in range(NQ)]
        pack_all = [dram.tile([NCORES * PACKQ], f8e3, tag=f"pk_all{qt}",
                              name=f"pk_all{qt}", addr_space="Shared")
                    for qt in range(NQ)]

        gcn_p = tc.tile_pool(name="gcn", bufs=1)
        gcn = gcn_p.__enter__()

        def wtile(h, chunks, width, name, pool=None):
            t = (pool or const).tile([128, chunks, width], bf16, tag=name)
            nc.sync.dma_start(
                t[:], h[:, :].rearrange("(c p) w -> p c w", p=128))
            return t

        def bcast_load(h, width, dt, name, pool=None):
            t = (pool or const).tile([128, width], dt, tag=name)
            bc = bass.AP(tensor=h.ap().tensor, offset=h.ap().offset,
                         ap=[[0, 128]] + list(h.ap().ap[1:]))
            nc.sync.dma_start(t[:], bc)
            return t

        hn_sb = gcn.tile([128, NCH, D], f8e4, tag="hn")
        hres_sb = gcn.tile([128, RB, D], f32, tag="hres")

        def hn_load(sl, eng):
            w = NCH * D // 4
            eng.dma_start(
                hn_sb[:].rearrange("p c d -> p (c d)")[:, sl * w:(sl + 1) * w],
                hn_h[:, sl * w:(sl + 1) * w])

        # startup: the first a_h needs hn slabs 0/1 + at(0,hf0) early;
        # slabs 1/2 ride behind the at(0) tiles on gpsimd/scalar
        hn_load(0, nc.sync)
        hn_load(3, nc.sync)
        nc.sync.dma_start(hres_sb[:, 0, :], hres_h[:, 0:D])
        wgcn_sb = wtile(wgcn_h, KC, D, "wgcn", gcn)
        gb_sb = {}
        for nm in ("g1", "be1"):
            gb_sb[nm] = bcast_load(gb_h[nm], D, bf16, nm, gcn)
        for rb in range(1, RB):
            nc.sync.dma_start(hres_sb[:, rb, :], hres_h[:, rb * D:(rb + 1) * D])
        wk_sb = wtile(wk_h, KC, D, "wk", gcn)
        wq_sb = wtile(wq_h, KC, D, "wq", gcn)
        wv_sb = wtile(wv_h, KC, D, "wv", gcn)
        rft_sb = wtile(rft_h, KC, M, "rft", gcn)
        bq_sb = gcn.tile([128, KC], f32, tag="bq")
        bk_sb = gcn.tile([128, KC], f32, tag="bk")
        for j in range(KC):
            nc.sync.dma_start(bq_sb[:, j:j + 1], bq_h[j * 128:(j + 1) * 128, :])
            nc.sync.dma_start(bk_sb[:, j:j + 1], bk_h[j * 128:(j + 1) * 128, :])
        bvr_bc = bcast_load(bvr_h, D, bf16, "bvr", gcn)
        ones_k1 = const.tile([1, 128], bf16, tag="ones")
        nc.vector.memset(ones_k1[:], 1.0)
        ident_bf = const.tile([128, 128], bf16, tag="ident")
        make_identity(nc, ident_bf[:])
        eps_t = const.tile([128, 1], f32, tag="eps")
        nc.vector.memset(eps_t[:], EPS)

        def load_p2_weights():
            w = {}
            w["wo"] = wtile(wo_h, KC, D, "wo")
            w["w1"] = wtile(w1_h, KC, F, "w1")
            w["w2"] = wtile(w2_h, FC, D, "w2")
            bor_r = const.tile([1, D], bf16, tag="bor")
            nc.sync.dma_start(bor_r[:], bor_h[:, :])
            w["bor"] = bor_r
            b1c_sb = const.tile([128, FC], f32, tag="b1c")
            for jf in range(FC):
                nc.sync.dma_start(b1c_sb[:, jf:jf + 1],
                                  b1c_h[jf * 128:(jf + 1) * 128, :])
            w["b1c"] = b1c_sb
            b2r_r = const.tile([1, D], bf16, tag="b2r")
            nc.sync.dma_start(b2r_r[:], b2r_h[:, :])
            w["b2r"] = b2r_r
            for nm in ("g2", "be2", "g3", "be3"):
                gb_sb[nm] = bcast_load(gb_h[nm], D, bf16, nm)
            return w

        def fire(qt):
            nc.gpsimd.collective_compute(
                "AllGather", mybir.AluOpType.bypass,
                replica_groups=[list(range(NCORES))],
                ins=[pack_loc[qt][:].opt()], outs=[pack_all[qt][:].opt()])

        def unpack(qt):
            # batched unpacks: 4 triggers per quarter instead of 24
            # (engine trigger time is ~0.6us apiece)
            base = pack_all[qt][:]
            for mc in range(KC):
                ksrc = bass.AP(
                    tensor=base.tensor,
                    offset=base.offset + mc * 128 * RQ,
                    ap=[[RQ, 128], [PACKQ, NCORES], [1, RQ]])
                kdst = kfaq[qt][:, mc, :].rearrange(
                    "p (c r) -> p c r", c=NCORES)
                # mc0 on sync, mc1 on gpsimd: both feature chunks land
                # together, so the first score chunk's j=1 never waits
                (nc.sync if mc == 0 else nc.gpsimd).dma_start(kdst, ksrc)
            for li in range(2):
                vsrc = bass.AP(
                    tensor=base.tensor,
                    offset=base.offset + KFTQ_B + li * 128 * VA,
                    ap=[[VA, 128], [PACKQ, NCORES], [1, VA]])
                vdst = vaugq[qt][:].rearrange(
                    "p (c l) v -> p c l v", c=NCORES)[:, :, li, :]
                nc.sync.dma_start(vdst, vsrc)

        # ============ Phase 1: GCN + kv quarters + q features ============
        kt_sb = gcn.tile([128, KC, R], bf16, tag="kt")
        qt_sb = gcn.tile([128, KC, R], bf16, tag="qt")
        h1t_sb = gcn.tile([128, KC, R], bf16, tag="h1t")
        with ExitStack() as p1:
            atp = p1.enter_context(tc.tile_pool(name="atp", bufs=8))
            sc1 = p1.enter_context(tc.tile_pool(name="sc1", bufs=4))
            mm_ps = p1.enter_context(tc.tile_pool(name="mm_ps", bufs=2, space="PSUM"))
            ah_ps = p1.enter_context(tc.tile_pool(name="ah_ps", bufs=3, space="PSUM"))
            tp_ps = p1.enter_context(tc.tile_pool(name="tp_ps", bufs=2, space="PSUM"))

            AT_Q = (nc.gpsimd, nc.scalar)

            def at_load(rb, split=False):
                tiles = []
                for hf in range(2):
                    at_t = atp.tile([128, NCH // 2, 128], f8e4, tag="at",
                                    name=f"at{rb}_{hf}")
                    q = AT_Q[hf]
                    if split:
                        # rb0: sub-DMAs so the first matmuls start sooner;
                        # the hn mid-slab rides between the halves
                        for s in range(2):
                            q.dma_start(at_t[:, s * 8:(s + 1) * 8, :],
                                        at_h[rb, hf, :, s * 8:(s + 1) * 8, :])
                        hn_load(1 + hf, q)
                        for s in range(2, 4):
                            q.dma_start(at_t[:, s * 8:(s + 1) * 8, :],
                                        at_h[rb, hf, :, s * 8:(s + 1) * 8, :])
                    else:
                        q.dma_start(at_t[:], at_h[rb, hf])
                    tiles.append(at_t)
                return tiles

            def a_h(rb, tiles):
                ps = ah_ps.tile([128, D], f32, tag="ah")
                for hf in range(2):
                    at_t = tiles[hf]
                    for k in range(NCH // 4):
                        pr = hf * (NCH // 4) + k
                        c0 = hf * (NCH // 2) + 2 * k
                        nc.tensor.matmul(
                            ps[:], at_t[:, 2 * k:2 * k + 2, :],
                            hn_sb[:, c0:c0 + 2, :],
                            start=(pr == 0), stop=(pr == NCH // 2 - 1),
                            perf_mode=DR)
                return ps

            ahb_t = {}
            h1bf_t = {}

            def post_s0(rb, ps):
                ahb = sc1.tile([128, D], bf16, tag="ahb")
                nc.scalar.activation(ahb[:], ps[:], Copy)
                ahb_t[rb] = ahb

            def post_s1(rb):
                ahb = ahb_t.pop(rb)
                ahT = sc1.tile([128, KC, 128], bf16, tag="ahT")
                for j in range(KC):
                    tp = tp_ps.tile([128, 128], bf16, tag="tp1")
                    nc.tensor.transpose(tp[:], ahb[:, j * 128:(j + 1) * 128],
                                        ident_bf[:])
                    nc.scalar.activation(ahT[:, j, :], tp[:], Copy)
                hl = mm_ps.tile([128, 512], f32, tag="mm")
                for j in range(KC):
                    nc.tensor.matmul(hl[:, 0:D], ahT[:, j, :], wgcn_sb[:, j, :],
                                     start=(j == 0), stop=(j == KC - 1))
                x1 = sc1.tile([128, D], f32, tag="x1")
                nc.vector.tensor_add(x1[:], hl[:, 0:D], hres_sb[:, rb, :])
                h1n = sc1.tile([128, D], f32, tag="h1n")
                _ln_block(nc, sc1, x1[:], h1n[:], eps_t=eps_t[:])
                nc.gpsimd.tensor_mul(h1_sb[:, rb, :], h1n[:], gb_sb["g1"][:])
                nc.gpsimd.tensor_add(h1_sb[:, rb, :], h1_sb[:, rb, :],
                                     gb_sb["be1"][:])
                h1bf = sc1.tile([128, D], bf16, tag="h1bf")
                nc.vector.tensor_copy(h1bf[:], h1n[:])
                h1bf_t[rb] = h1bf

            def post_s2(rb):
                h1bf = h1bf_t.pop(rb)
                for j in range(KC):
                    tp = tp_ps.tile([128, 128], bf16, tag="tp1")
                    nc.tensor.transpose(tp[:], h1bf[:, j * 128:(j + 1) * 128],
                                        ident_bf[:])
                    nc.vector.tensor_copy(
                        h1t_sb[:, j, rb * 128:(rb + 1) * 128], tp[:])

            def vrow(rb, qt, li):
                ps = mm_ps.tile([128, 512], f32, tag="mm")
                for j in range(KC):
                    nc.tensor.matmul(ps[:, 0:D],
                                     h1t_sb[:, j, rb * 128:(rb + 1) * 128],
                                     wv_sb[:, j, :],
                                     start=(j == 0), stop=(j == KC - 1))
                vt = sc1.tile([128, VA], f8e3, tag="vaug")
                nc.vector.tensor_add(vt[:, 0:D], ps[:, 0:D], bvr_bc[:])
                nc.vector.memset(vt[:, D:D + 1], 1.0)
                nc.vector.memset(vt[:, D + 1:VA], 0.0)
                off = KFTQ_B + li * 128 * VA
                nc.sync.dma_start(
                    pack_loc[qt][off:off + 128 * VA].rearrange(
                        "(p v) -> p v", p=128),
                    vt[:])

            def kv_quarter(qt):
                r0 = qt * RQ
                for jj in range(KC):
                    ps = mm_ps.tile([128, 512], f32, tag="mm")
                    for j in range(KC):
                        nc.tensor.matmul(
                            ps[:, 0:RQ],
                            wk_sb[:, j, jj * 128:(jj + 1) * 128],
                            h1t_sb[:, j, r0:r0 + RQ],
                            start=(j == 0), stop=(j == KC - 1))
                    nc.vector.tensor_scalar(
                        out=kt_sb[:, jj, r0:r0 + RQ], in0=ps[:, 0:RQ],
                        scalar1=bk_sb[:, jj:jj + 1], scalar2=None, op0=ADD)
                for mc in range(KC):
                    ps = mm_ps.tile([128, 512], f32, tag="mm")
                    for j in range(KC):
                        nc.tensor.matmul(
                            ps[:, 0:RQ],
                            rft_sb[:, j, mc * 128:(mc + 1) * 128],
                            kt_sb[:, j, r0:r0 + RQ],
                            start=(j == 0), stop=(j == KC - 1))
                    kq = sc1.tile([128, RQ], f8e3, tag="kftq")
                    nc.vector.tensor_copy(kq[:], ps[:, 0:RQ])
                    nc.sync.dma_start(
                        pack_loc[qt][mc * 128 * RQ:(mc + 1) * 128 * RQ]
                        .rearrange("(p r) -> p r", p=128), kq[:])
                vrow(2 * qt + 1, qt, 1)
                fire(qt)

            def qside(hf):
                RH = R // 2
                r0 = hf * RH
                for jj in range(KC):
                    ps = mm_ps.tile([128, 512], f32, tag="mm")
                    for j in range(KC):
                        nc.tensor.matmul(
                            ps[:],
                            wq_sb[:, j, jj * 128:(jj + 1) * 128],
                            h1t_sb[:, j, r0:r0 + RH],
                            start=(j == 0), stop=(j == KC - 1))
                    nc.vector.tensor_scalar(
                        out=qt_sb[:, jj, r0:r0 + RH], in0=ps[:],
                        scalar1=bq_sb[:, jj:jj + 1], scalar2=None, op0=ADD)
                for mc in range(KC):
                    ps = mm_ps.tile([128, 512], f32, tag="mm")
                    for j in range(KC):
                        nc.tensor.matmul(
                            ps[:],
                            rft_sb[:, j, mc * 128:(mc + 1) * 128],
                            qt_sb[:, j, r0:r0 + RH],
                            start=(j == 0), stop=(j == KC - 1))
                    nc.vector.tensor_copy(qft_sb[:, mc, r0:r0 + RH], ps[:])

            # GCN row blocks; kv quarter fired every 2 blocks
            tiles = at_load(0, split=True)
            nxt = at_load(1)
            ps = a_h(0, tiles); post_s0(0, ps)
            tiles, nxt = nxt, at_load(2)
            ps = a_h(1, tiles); post_s0(1, ps); post_s1(0)
            tiles, nxt = nxt, at_load(3)
            ps = a_h(2, tiles); post_s0(2, ps); post_s1(1); post_s2(0)
            vrow(0, 0, 0); post_s2(1); kv_quarter(0)
            tiles, nxt = nxt, at_load(4)
            ps = a_h(3, tiles); post_s0(3, ps); post_s1(2)
            tiles, nxt = nxt, at_load(5)
            ps = a_h(4, tiles); post_s0(4, ps); post_s1(3); post_s2(2)
            vrow(2, 1, 0); post_s2(3); kv_quarter(1)
            tiles, nxt = nxt, at_load(6)
            ps = a_h(5, tiles); post_s0(5, ps); post_s1(4)
            tiles, nxt = nxt, at_load(7)
            ps = a_h(6, tiles); post_s0(6, ps); post_s1(5); post_s2(4)
            vrow(4, 2, 0); post_s2(5); kv_quarter(2)
            tiles = nxt
            ps = a_h(7, tiles); post_s0(7, ps); post_s1(6); post_s2(6)
            vrow(6, 3, 0); post_s1(7); post_s2(7); kv_quarter(3)

            unpack(0)
            p2w = load_p2_weights()
            qside(0)
            qside(1)
            wo_sb, w1_sb, w2_sb = p2w["wo"], p2w["w1"], p2w["w2"]
            bor_r, b1c_sb, b2r_r = p2w["bor"], p2w["b1c"], p2w["b2r"]

        gcn_p.__exit__(None, None, None)

        # ============ Phase 2: attention + FFN ============
        with ExitStack() as p3:
            slabs = p3.enter_context(tc.tile_pool(name="slabs", bufs=2))
            pva_p = p3.enter_context(tc.tile_pool(name="pva", bufs=1))
            sc3 = p3.enter_context(tc.tile_pool(name="sc3", bufs=2))
            sc4 = p3.enter_context(tc.tile_pool(name="sc4", bufs=3))
            st_ps = p3.enter_context(tc.tile_pool(name="st_ps", bufs=2, space="PSUM"))
            num_ps = p3.enter_context(tc.tile_pool(name="num_ps", bufs=1, space="PSUM"))
            tp2_ps = p3.enter_context(tc.tile_pool(name="tp2_ps", bufs=1, space="PSUM"))
            acc_ps = p3.enter_context(tc.tile_pool(name="acc_ps", bufs=2, space="PSUM"))

            RC = 512  # rows per score slab (2 slabs cover R=1024)
            NSL = R // RC

            slab = [slabs.tile([128, NCH, RC], bf16, tag="slab",
                               name=f"slab{i}")
                    for i in range(NSL)]
            pva_sb = pva_p.tile([128, RB, VA], bf16, tag="pva")

            def scores_q(rc, qt):
                """Scores + exp for one quarter's 16 chunks, exp batched
                2 chunks per ACTIVATE."""
                for c in range(NCORES):
                    cg0 = c * 8 + 2 * qt
                    ps = st_ps.tile([128, 2, RC], f32, tag="st")
                    for t in range(2):
                        for j in range(KC):
                            nc.tensor.matmul(
                                ps[:, t, :],
                                kfaq[qt][:, j, (2 * c + t) * 128:
                                         (2 * c + t + 1) * 128],
                                qft_sb[:, j, rc * RC:(rc + 1) * RC],
                                start=(j == 0), stop=(j == KC - 1))
                    nc.scalar.activation(slab[rc][:, cg0:cg0 + 2, :], ps[:],
                                         Exp, scale=1.0 / 16.0)

            def pv_half(rc, hb, hf):
                """P@V accumulation for one half (quarters 2h, 2h+1)."""
                nps = num_ps.tile([128, VA], f32, tag="num")
                n = 0
                for qt in (2 * hf, 2 * hf + 1):
                    for c in range(NCORES):
                        for t in range(2):
                            cg = c * 8 + 2 * qt + t
                            nc.tensor.matmul(
                                nps[:],
                                slab[rc][:, cg, hb * 128:(hb + 1) * 128],
                                vaugq[qt][:, 2 * c + t, :],
                                start=(n == 0), stop=(n == 31))
                            n += 1
                return nps

            def pv_a(rc):
                for hb in range(RC // 128):
                    rb = rc * (RC // 128) + hb
                    nps = pv_half(rc, hb, 0)
                    nc.vector.tensor_copy(pva_sb[:, rb, :], nps[:])

            def attn_stage(rc, hb):
                rb = rc * (RC // 128) + hb
                nps = pv_half(rc, hb, 1)
                xf = sc3.tile([128, VA], f32, tag="xf")
                nc.vector.tensor_add(xf[:], nps[:], pva_sb[:, rb, :])
                rec = sc3.tile([128, 1], f32, tag="rec")
                nc.vector.reciprocal(rec[:], xf[:, D:D + 1])
                attn_bf = sc3.tile([128, D], bf16, tag="attn")
                nc.vector.tensor_scalar(out=attn_bf[:], in0=xf[:, 0:D],
                                        scalar1=rec[:, 0:1], scalar2=None,
                                        op0=mybir.AluOpType.mult)
                attnT = sc3.tile([128, KC, 128], bf16, tag="attnT")
                for j in range(KC):
                    tp = tp2_ps.tile([128, 128], bf16, tag="tp2")
                    nc.tensor.transpose(
                        tp[:], attn_bf[:, j * 128:(j + 1) * 128], ident_bf[:])
                    nc.vector.tensor_copy(attnT[:, j, :], tp[:])
                hg = acc_ps.tile([128, D], f32, tag="acc")
                for j in range(KC):
                    nc.tensor.matmul(hg[:], attnT[:, j, :], wo_sb[:, j, :],
                                     start=(j == 0), stop=False)
                nc.tensor.matmul(hg[:], ones_k1[:], bor_r[:],
                                 start=False, stop=True)
                x2 = sc3.tile([128, D], f32, tag="x2")
                nc.vector.tensor_add(x2[:], hg[:], h1_sb[:, rb, :])
                h2n = sc4.tile([128, D], f32, tag="h2n")
                _ln_block(nc, sc3, x2[:], h2n[:], eps_t=eps_t[:])
                h2 = sc4.tile([128, D], f32, tag="h2")
                nc.gpsimd.tensor_mul(h2[:], h2n[:], gb_sb["g2"][:])
                nc.gpsimd.tensor_add(h2[:], h2[:], gb_sb["be2"][:])
                h2bf = sc4.tile([128, D], bf16, tag="h2bf")
                nc.vector.tensor_copy(h2bf[:], h2n[:])
                return rb, h2, h2bf

            def ffn_stage(st):
                rb, h2, h2bf = st
                h2T = sc3.tile([128, KC, 128], bf16, tag="h2T")
                for j in range(KC):
                    tp = tp2_ps.tile([128, 128], bf16, tag="tp2")
                    nc.tensor.transpose(
                        tp[:], h2bf[:, j * 128:(j + 1) * 128], ident_bf[:])
                    nc.vector.tensor_copy(h2T[:, j, :], tp[:])
                uT = sc3.tile([128, FC, 128], bf16, tag="uT")
                for jf in range(FC):
                    up = acc_ps.tile([128, D], f32, tag="acc")
                    for j in range(KC):
                        nc.tensor.matmul(
                            up[:, 0:128],
                            w1_sb[:, j, jf * 128:(jf + 1) * 128],
                            h2T[:, j, :],
                            start=(j == 0), stop=(j == KC - 1))
                    nc.vector.tensor_scalar(
                        out=uT[:, jf, :], in0=up[:, 0:128],
                        scalar1=b1c_sb[:, jf:jf + 1], scalar2=0.0,
                        op0=mybir.AluOpType.add,
                        op1=mybir.AluOpType.max)
                o2 = acc_ps.tile([128, D], f32, tag="acc")
                for jf in range(FC):
                    nc.tensor.matmul(o2[:], uT[:, jf, :], w2_sb[:, jf, :],
                                     start=(jf == 0), stop=False)
                nc.tensor.matmul(o2[:], ones_k1[:], b2r_r[:],
                                 start=False, stop=True)
                x3 = sc3.tile([128, D], f32, tag="x3")
                nc.vector.tensor_add(x3[:], o2[:], h2[:])
                o_sb = sc3.tile([128, D], f32, tag="osb")
                _ln_block(nc, sc3, x3[:], o_sb[:],
                          gb_sb["g3"][:], gb_sb["be3"][:], eps_t[:],
                          tail=nc.gpsimd)
                nc.sync.dma_start(out_h[rb * 128:(rb + 1) * 128, :], o_sb[:])

            def pv_b_all():
                # one continuous 2-stage pipeline over all 8 row blocks,
                # with the last score group interleaved at the slab
                # boundary so the pipeline never drains mid-way
                prev = None
                for hb in range(RC // 128):
                    st = attn_stage(0, hb)
                    if prev is not None:
                        ffn_stage(prev)
                    prev = st
                scores_q(1, 3)
                for hb in range(RC // 128):
                    st = attn_stage(1, hb)
                    ffn_stage(prev)
                    prev = st
                ffn_stage(prev)

            # arrival order at quarter granularity; later quarters'
            # unpack triggers are staged between score groups so the sync
            # engine's semaphore waits overlap PE work on earlier quarters
            scores_q(0, 0)
            unpack(1)
            scores_q(1, 0)
            scores_q(0, 1)
            unpack(2)
            scores_q(1, 1)
            pv_a(0)
            pv_a(1)
            scores_q(0, 2)
            unpack(3)
            scores_q(1, 2)
            scores_q(0, 3)
            pv_b_all()


_NC_CACHE = None


def _get_nc():
    global _NC_CACHE
    if _NC_CACHE is None:
        _NC_CACHE = _build()
    return _NC_CACHE


def _host_prep(inputs):
    """Build per-core in_maps from full inputs."""
    h = np.ascontiguousarray(np.asarray(inputs["h"], dtype=np.float32))
    ei = np.asarray(inputs["edge_index"]).astype(np.int64)
    src, dst = ei[0], ei[1]

    deg = np.bincount(dst, minlength=N).astype(np.float32) + 1.0
    dinv = 1.0 / np.sqrt(deg)
    coef = (dinv[src] * dinv[dst]).astype(np.float32)
    A = np.zeros((N, N), np.float32)
    np.add.at(A, (dst, src), coef)
    idx = np.arange(N)
    A[idx, idx] += dinv * dinv

    f32c = lambda k: np.ascontiguousarray(np.asarray(inputs[k], dtype=np.float32))
    bfc = lambda x: np.ascontiguousarray(x.astype(BF))

    w = {k: f32c(k) for k in ("W_gcn", "Wq", "Wk", "Wv", "Wo", "RF",
                              "W1", "W2", "b_gcn", "bq", "bk", "bv", "bo",
                              "b1", "b2", "g1", "be1", "g2", "be2", "g3", "be3")}

    hn = np.ascontiguousarray(
        h.reshape(NCH, 128, D).transpose(1, 0, 2).reshape(128, NCH * D)
        .astype(E4))

    # fold the layernorm affines into the consuming projections
    w1f = w["W1"] * w["g2"].reshape(D, 1)
    b1f = w["b1"] + w["be2"] @ w["W1"]
    g1c = w["g1"].reshape(D, 1)
    wqf = w["Wq"] * g1c
    wkf = w["Wk"] * g1c
    wvf = w["Wv"] * g1c
    bqf = w["bq"] + w["be1"] @ w["Wq"]
    bkf = w["bk"] + w["be1"] @ w["Wk"]
    bvf = w["bv"] + w["be1"] @ w["Wv"]

    common = {
        "hn": hn,
        "wgcn": bfc(w["W_gcn"]), "wq": bfc(wqf), "wk": bfc(wkf),
        "wv": bfc(wvf), "wo": bfc(w["Wo"]), "rft": bfc(w["RF"].T),
        "w1": bfc(w1f), "w2": bfc(w["W2"]),
        "bqc": np.ascontiguousarray(bqf.reshape(D, 1)),
        "bkc": np.ascontiguousarray(bkf.reshape(D, 1)),
        "b1c": np.ascontiguousarray(b1f.reshape(F, 1)),
        "bvr": bfc(bvf.reshape(1, D)),
        "bor": bfc(w["bo"].reshape(1, D)),
        "b2r": bfc(w["b2"].reshape(1, D)),
        "g1": bfc(w["g1"].reshape(1, D)),
        "be1": bfc(w["be1"].reshape(1, D)),
        "g2": bfc(w["g2"].reshape(1, D)),
        "be2": bfc(w["be2"].reshape(1, D)),
        "g3": bfc(w["g3"].reshape(1, D)),
        "be3": bfc(w["be3"].reshape(1, D)),
    }

    in_maps = []
    for c in range(NCORES):
        r0 = c * R
        a_loc = A[r0:r0 + R].reshape(RB, 128, 2, NCH // 2, 128)
        at = np.ascontiguousarray(a_loc.transpose(0, 2, 4, 3, 1).astype(E4))
        hr = (h[r0:r0 + R] + w["b_gcn"]).reshape(RB, 128, D).transpose(
            1, 0, 2).reshape(128, RB * D)
        m = dict(common)
        m["at"] = at
        m["hres"] = np.ascontiguousarray(hr)
        in_maps.append(m)
    return in_maps


def kernel(**inputs):
    nc = _get_nc()
    in_maps = _host_prep(inputs)
    res = run_bass_kernel_spmd(nc, in_maps, core_ids=list(range(NCORES)))
    out = np.concatenate([np.asarray(r["out"]) for r in res.results], axis=0)
    return out.astype(np.float32)


# revision 32
# speedup vs baseline: 1.0429x; 1.0429x over previous
"""GPS layer (GCN + dense Performer attention + FFN) on 8 Trainium2 cores.

Strategy (per core, rows R=1024 of N=8192 nodes):
  - GCN segment-sum as a dense matmul with the normalized adjacency
    A = D^-1/2 (Adj + I) D^-1/2, built host-side, shipped fp8-e4m3 in an
    lhsT-friendly layout, computed as (A @ h) @ W_gcn with h also e4m3 so
    the big matmul runs in DoubleRow mode (2 moving pixels/cycle).
  - Attention in transposed score layout ST[c, r] = kf@qf^T: softmax via
    the appended ones-column of V (no max subtraction needed in bf16),
    exp(ST) tiles are directly the lhsT of the P@V matmul.  qf is bf16
    (local only), kf/v fp8-e3m4 on the wire.
  - k-features and V are all-gathered in QUARTER payloads (132KB/rank,
    4 collectives) fired as soon as each pair of GCN row blocks clears,
    so the gather pipeline runs behind the remaining GCN work and the
    first payload lands before phase 2 starts.  kfa/vaug live in
    per-quarter SBUF tiles so a score chunk only depends on its own
    quarter's unpack (tile-granular dependency tracking).  Pack writes
    ride the scalar queue; kfa unpacks the sync ring, vaug the gpsimd
    ring -- an unpack can never sit behind a later quarter's pack.
  - Phase 2 in arrival order at quarter granularity: scores+exp per
    quarter, half-A P@V partial saved to SBUF, half-B P@V + output
    projection + FFN per slab.
"""

import os
import sys

sys.path.insert(0, "/opt/trn_rl_repo")
os.environ.setdefault("MYCRO_LOCAL_CACHE", "1")

import numpy as np
import ml_dtypes

import concourse.bass as bass
import concourse.tile as tile
from concourse import bacc, mybir
from concourse.bass_utils import run_bass_kernel_spmd
from concourse.masks import make_identity

f32 = mybir.dt.float32
bf16 = mybir.dt.bfloat16
f8e3 = mybir.dt.float8e3
f8e4 = mybir.dt.float8e4
DR = mybir.MatmulPerfMode.DoubleRow
BF = ml_dtypes.bfloat16
E3 = ml_dtypes.float8_e3m4
E4 = ml_dtypes.float8_e4m3

N, D, F, M = 8192, 256, 512, 256
NCORES = 8
R = N // NCORES          # rows per core (1024)
RB = R // 128            # row blocks per core (8)
KC = D // 128            # feature chunks (2)
NCH = N // 128           # node chunks (64)
FC = F // 128            # ffn chunks (4)
VA = 260                 # v free dim: 256 features + ones col + pad
EPS = 1e-5
NQ = 4                   # collective quarters
RQ = R // NQ             # rows per quarter (256)
KFTQ_B = M * RQ          # kft bytes per quarter (e3m4): 65536
VAUGQ_B = RQ * VA        # vaug bytes per quarter (e3m4): 66560
PACKQ = KFTQ_B + VAUGQ_B # 132096 bytes per rank per quarter


def _ln_block(nc, pool, x_sb, out_sb, g_bc=None, be_bc=None, eps_t=None,
              tail=None):
    """LayerNorm over free dim (256) of x_sb [128, 256] f32 -> out_sb."""
    tail = tail or nc.vector
    stats = pool.tile([128, 6], f32, tag="ln_stats")
    nc.vector.bn_stats(stats[:], x_sb)
    mv = pool.tile([128, 2], f32, tag="ln_mv")
    nc.vector.bn_aggr(mv[:], stats[:])
    nc.scalar.activation(mv[:, 1:2], mv[:, 1:2],
                         mybir.ActivationFunctionType.Sqrt, bias=eps_t)
    nc.vector.reciprocal(mv[:, 1:2], mv[:, 1:2])
    nc.vector.tensor_scalar(out=out_sb, in0=x_sb,
                            scalar1=mv[:, 0:1], scalar2=mv[:, 1:2],
                            op0=mybir.AluOpType.subtract,
                            op1=mybir.AluOpType.mult)
    if g_bc is not None:
        tail.tensor_mul(out_sb, out_sb, g_bc)
        tail.tensor_add(out_sb, out_sb, be_bc)


def _build():
    nc = bacc.Bacc("TRN2", target_bir_lowering=False, debug=False,
                   num_devices=NCORES)

    def inp(name, shape, dt):
        return nc.dram_tensor(name, shape, dt, kind="ExternalInput")

    at_h = inp("at", [RB, 2, 128, NCH // 2, 128], f8e4)
    hn_h = inp("hn", [128, NCH * D], f8e4)
    hres_h = inp("hres", [128, RB * D], f32)
    wgcn_h = inp("wgcn", [D, D], bf16)
    wq_h = inp("wq", [D, D], bf16)
    wk_h = inp("wk", [D, D], bf16)
    wv_h = inp("wv", [D, D], bf16)
    wo_h = inp("wo", [D, D], bf16)
    rft_h = inp("rft", [D, M], bf16)
    w1_h = inp("w1", [D, F], bf16)
    w2_h = inp("w2", [F, D], bf16)
    bq_h = inp("bqc", [D, 1], f32)
    bk_h = inp("bkc", [D, 1], f32)
    b1c_h = inp("b1c", [F, 1], f32)
    bvr_h = inp("bvr", [1, D], bf16)
    bor_h = inp("bor", [1, D], bf16)
    b2r_h = inp("b2r", [1, D], bf16)
    gb_h = {}
    for nm in ("g1", "be1", "g2", "be2", "g3", "be3"):
        gb_h[nm] = inp(nm, [1, D], bf16)

    out_h = nc.dram_tensor("out", [R, D], f32, kind="ExternalOutput")

    with tile.TileContext(nc) as tc:
        _body(tc, at_h, hn_h, hres_h, wgcn_h, wq_h, wk_h, wv_h, wo_h, rft_h,
              w1_h, w2_h, bq_h, bk_h, b1c_h, bvr_h, bor_h, b2r_h, gb_h, out_h)
    nc.compile()
    return nc


def _body(tc, at_h, hn_h, hres_h, wgcn_h, wq_h, wk_h, wv_h, wo_h, rft_h,
          w1_h, w2_h, bq_h, bk_h, b1c_h, bvr_h, bor_h, b2r_h, gb_h, out_h):
    from contextlib import ExitStack
    nc = tc.nc
    Exp = mybir.ActivationFunctionType.Exp
    Copy = mybir.ActivationFunctionType.Copy
    Relu = mybir.ActivationFunctionType.Relu
    ADD = mybir.AluOpType.add

    with ExitStack() as octx:
        const = octx.enter_context(tc.tile_pool(name="const", bufs=1))
        persist = octx.enter_context(tc.tile_pool(name="persist", bufs=1))
        dram = octx.enter_context(tc.tile_pool(name="dram", bufs=1, space="DRAM"))

        # ---- persistent activations (per-quarter kfa/vaug tiles) ----
        kfaq = [persist.tile([128, KC, NCORES * RQ], f8e3, tag=f"kfaq{qt}",
                             name=f"kfaq{qt}") for qt in range(NQ)]
        vaugq = [persist.tile([128, NCORES * 2, VA], f8e3, tag=f"vaugq{qt}",
                              name=f"vaugq{qt}") for qt in range(NQ)]
        h1_sb = persist.tile([128, RB, D], f32, tag="h1")
        qft_sb = persist.tile([128, KC, R], bf16, tag="qft")

        # ---- packed collective DRAM buffers (four quarters) ----
        pack_loc = [dram.tile([PACKQ], f8e3, tag=f"pk_loc{qt}",
                              name=f"pk_loc{qt}") for qt in range(NQ)]
        pack_all = [dram.tile([NCORES * PACKQ], f8e3, tag=f"pk_all{qt}",
                              name=f"pk_all{qt}", addr_space="Shared")
                    for qt in range(NQ)]

        gcn_p = tc.tile_pool(name="gcn", bufs=1)
        gcn = gcn_p.__enter__()

        def wtile(h, chunks, width, name, pool=None):
            t = (pool or const).tile([128, chunks, width], bf16, tag=name)
            nc.sync.dma_start(
                t[:], h[:, :].rearrange("(c p) w -> p c w", p=128))
            return t

        def bcast_load(h, width, dt, name, pool=None):
            t = (pool or const).tile([128, width], dt, tag=name)
            bc = bass.AP(tensor=h.ap().tensor, offset=h.ap().offset,
                         ap=[[0, 128]] + list(h.ap().ap[1:]))
            nc.sync.dma_start(t[:], bc)
            return t

        hn_sb = gcn.tile([128, NCH, D], f8e4, tag="hn")
        hres_sb = gcn.tile([128, RB, D], f32, tag="hres")

        def hn_load(sl, eng):
            w = NCH * D // 4
            eng.dma_start(
                hn_sb[:].rearrange("p c d -> p (c d)")[:, sl * w:(sl + 1) * w],
                hn_h[:, sl * w:(sl + 1) * w])

        # startup: the first a_h needs hn slabs 0/1 + at(0,hf0) early;
        # slabs 1/2 ride behind the at(0) tiles on gpsimd/scalar
        hn_load(0, nc.sync)
        hn_load(3, nc.sync)
        nc.sync.dma_start(hres_sb[:, 0, :], hres_h[:, 0:D])
        wgcn_sb = wtile(wgcn_h, KC, D, "wgcn", gcn)
        gb_sb = {}
        for nm in ("g1", "be1"):
            gb_sb[nm] = bcast_load(gb_h[nm], D, bf16, nm, gcn)
        for rb in range(1, RB):
            nc.sync.dma_start(hres_sb[:, rb, :], hres_h[:, rb * D:(rb + 1) * D])
        wk_sb = wtile(wk_h, KC, D, "wk", gcn)
        wq_sb = wtile(wq_h, KC, D, "wq", gcn)
        wv_sb = wtile(wv_h, KC, D, "wv", gcn)
        rft_sb = wtile(rft_h, KC, M, "rft", gcn)
        bq_sb = gcn.tile([128, KC], f32, tag="bq")
        bk_sb = gcn.tile([128, KC], f32, tag="bk")
        for j in range(KC):
            nc.sync.dma_start(bq_sb[:, j:j + 1], bq_h[j * 128:(j + 1) * 128, :])
            nc.sync.dma_start(bk_sb[:, j:j + 1], bk_h[j * 128:(j + 1) * 128, :])
        bvr_bc = bcast_load(bvr_h, D, bf16, "bvr", gcn)
        ones_k1 = const.tile([1, 128], bf16, tag="ones")
        nc.vector.memset(ones_k1[:], 1.0)
        ident_bf = const.tile([128, 128], bf16, tag="ident")
        make_identity(nc, ident_bf[:])
        eps_t = const.tile([128, 1], f32, tag="eps")
        nc.vector.memset(eps_t[:], EPS)

        def load_p2_weights():
            w = {}
            w["wo"] = wtile(wo_h, KC, D, "wo")
            w["w1"] = wtile(w1_h, KC, F, "w1")
            w["w2"] = wtile(w2_h, FC, D, "w2")
            bor_r = const.tile([1, D], bf16, tag="bor")
            nc.sync.dma_start(bor_r[:], bor_h[:, :])
            w["bor"] = bor_r
            b1c_sb = const.tile([128, FC], f32, tag="b1c")
            for jf in range(FC):
                nc.sync.dma_start(b1c_sb[:, jf:jf + 1],
                                  b1c_h[jf * 128:(jf + 1) * 128, :])
            w["b1c"] = b1c_sb
            b2r_r = const.tile([1, D], bf16, tag="b2r")
            nc.sync.dma_start(b2r_r[:], b2r_h[:, :])
            w["b2r"] = b2r_r
            for nm in ("g2", "be2", "g3", "be3"):
                gb_sb[nm] = bcast_load(gb_h[nm], D, bf16, nm)
            return w

        def fire(qt):
            nc.gpsimd.collective_compute(
                "AllGather", mybir.AluOpType.bypass,
                replica_groups=[list(range(NCORES))],
                ins=[pack_loc[qt][:].opt()], outs=[pack_all[qt][:].opt()])

        def unpack(qt):
            # batched unpacks: 4 triggers per quarter instead of 24
            # (engine trigger time is ~0.6us apiece)
            base = pack_all[qt][:]
            for mc in range(KC):
                ksrc = bass.AP(
                    tensor=base.tensor,
                    offset=base.offset + mc * 128 * RQ,
                    ap=[[RQ, 128], [PACKQ, NCORES], [1, RQ]])
                kdst = kfaq[qt][:, mc, :].rearrange(
                    "p (c r) -> p c r", c=NCORES)
                nc.sync.dma_start(kdst, ksrc)
            for li in range(2):
                vsrc = bass.AP(
                    tensor=base.tensor,
                    offset=base.offset + KFTQ_B + li * 128 * VA,
                    ap=[[VA, 128], [PACKQ, NCORES], [1, VA]])
                vdst = vaugq[qt][:].rearrange(
                    "p (c l) v -> p c l v", c=NCORES)[:, :, li, :]
                nc.sync.dma_start(vdst, vsrc)

        # ============ Phase 1: GCN + kv quarters + q features ============
        kt_sb = gcn.tile([128, KC, R], bf16, tag="kt")
        qt_sb = gcn.tile([128, KC, R], bf16, tag="qt")
        h1t_sb = gcn.tile([128, KC, R], bf16, tag="h1t")
        with ExitStack() as p1:
            atp = p1.enter_context(tc.tile_pool(name="atp", bufs=8))
            sc1 = p1.enter_context(tc.tile_pool(name="sc1", bufs=4))
            mm_ps = p1.enter_context(tc.tile_pool(name="mm_ps", bufs=2, space="PSUM"))
            ah_ps = p1.enter_context(tc.tile_pool(name="ah_ps", bufs=3, space="PSUM"))
            tp_ps = p1.enter_context(tc.tile_pool(name="tp_ps", bufs=2, space="PSUM"))

            AT_Q = (nc.gpsimd, nc.scalar)

            def at_load(rb, split=False):
                tiles = []
                for hf in range(2):
                    at_t = atp.tile([128, NCH // 2, 128], f8e4, tag="at",
                                    name=f"at{rb}_{hf}")
                    q = AT_Q[hf]
                    if split:
                        # rb0: sub-DMAs so the first matmuls start sooner;
                        # the hn mid-slab rides between the halves
                        for s in range(2):
                            q.dma_start(at_t[:, s * 8:(s + 1) * 8, :],
                                        at_h[rb, hf, :, s * 8:(s + 1) * 8, :])
                        hn_load(1 + hf, q)
                        for s in range(2, 4):
                            q.dma_start(at_t[:, s * 8:(s + 1) * 8, :],
                                        at_h[rb, hf, :, s * 8:(s + 1) * 8, :])
                    else:
                        q.dma_start(at_t[:], at_h[rb, hf])
                    tiles.append(at_t)
                return tiles

            def a_h(rb, tiles):
                ps = ah_ps.tile([128, D], f32, tag="ah")
                for hf in range(2):
                    at_t = tiles[hf]
                    for k in range(NCH // 4):
                        pr = hf * (NCH // 4) + k
                        c0 = hf * (NCH // 2) + 2 * k
                        nc.tensor.matmul(
                            ps[:], at_t[:, 2 * k:2 * k + 2, :],
                            hn_sb[:, c0:c0 + 2, :],
                            start=(pr == 0), stop=(pr == NCH // 2 - 1),
                            perf_mode=DR)
                return ps

            ahb_t = {}
            h1bf_t = {}

            def post_s0(rb, ps):
                ahb = sc1.tile([128, D], bf16, tag="ahb")
                nc.scalar.activation(ahb[:], ps[:], Copy)
                ahb_t[rb] = ahb

            def post_s1(rb):
                ahb = ahb_t.pop(rb)
                ahT = sc1.tile([128, KC, 128], bf16, tag="ahT")
                for j in range(KC):
                    tp = tp_ps.tile([128, 128], bf16, tag="tp1")
                    nc.tensor.transpose(tp[:], ahb[:, j * 128:(j + 1) * 128],
                                        ident_bf[:])
                    nc.scalar.activation(ahT[:, j, :], tp[:], Copy)
                hl = mm_ps.tile([128, 512], f32, tag="mm")
                for j in range(KC):
                    nc.tensor.matmul(hl[:, 0:D], ahT[:, j, :], wgcn_sb[:, j, :],
                                     start=(j == 0), stop=(j == KC - 1))
                x1 = sc1.tile([128, D], f32, tag="x1")
                nc.vector.tensor_add(x1[:], hl[:, 0:D], hres_sb[:, rb, :])
                h1n = sc1.tile([128, D], f32, tag="h1n")
                _ln_block(nc, sc1, x1[:], h1n[:], eps_t=eps_t[:])
                nc.gpsimd.tensor_mul(h1_sb[:, rb, :], h1n[:], gb_sb["g1"][:])
                nc.gpsimd.tensor_add(h1_sb[:, rb, :], h1_sb[:, rb, :],
                                     gb_sb["be1"][:])
                h1bf = sc1.tile([128, D], bf16, tag="h1bf")
                nc.vector.tensor_copy(h1bf[:], h1n[:])
                h1bf_t[rb] = h1bf

            def post_s2(rb):
                h1bf = h1bf_t.pop(rb)
                for j in range(KC):
                    tp = tp_ps.tile([128, 128], bf16, tag="tp1")
                    nc.tensor.transpose(tp[:], h1bf[:, j * 128:(j + 1) * 128],
                                        ident_bf[:])
                    nc.vector.tensor_copy(
                        h1t_sb[:, j, rb * 128:(rb + 1) * 128], tp[:])

            def vrow(rb, qt, li):
                ps = mm_ps.tile([128, 512], f32, tag="mm")
                for j in range(KC):
                    nc.tensor.matmul(ps[:, 0:D],
                                     h1t_sb[:, j, rb * 128:(rb + 1) * 128],
                                     wv_sb[:, j, :],
                                     start=(j == 0), stop=(j == KC - 1))
                vt = sc1.tile([128, VA], f8e3, tag="vaug")
                nc.vector.tensor_add(vt[:, 0:D], ps[:, 0:D], bvr_bc[:])
                nc.vector.memset(vt[:, D:D + 1], 1.0)
                nc.vector.memset(vt[:, D + 1:VA], 0.0)
                off = KFTQ_B + li * 128 * VA
                nc.sync.dma_start(
                    pack_loc[qt][off:off + 128 * VA].rearrange(
                        "(p v) -> p v", p=128),
                    vt[:])

            def kv_quarter(qt):
                r0 = qt * RQ
                for jj in range(KC):
                    ps = mm_ps.tile([128, 512], f32, tag="mm")
                    for j in range(KC):
                        nc.tensor.matmul(
                            ps[:, 0:RQ],
                            wk_sb[:, j, jj * 128:(jj + 1) * 128],
                            h1t_sb[:, j, r0:r0 + RQ],
                            start=(j == 0), stop=(j == KC - 1))
                    nc.vector.tensor_scalar(
                        out=kt_sb[:, jj, r0:r0 + RQ], in0=ps[:, 0:RQ],
                        scalar1=bk_sb[:, jj:jj + 1], scalar2=None, op0=ADD)
                for mc in range(KC):
                    ps = mm_ps.tile([128, 512], f32, tag="mm")
                    for j in range(KC):
                        nc.tensor.matmul(
                            ps[:, 0:RQ],
                            rft_sb[:, j, mc * 128:(mc + 1) * 128],
                            kt_sb[:, j, r0:r0 + RQ],
                            start=(j == 0), stop=(j == KC - 1))
                    kq = sc1.tile([128, RQ], f8e3, tag="kftq")
                    nc.vector.tensor_copy(kq[:], ps[:, 0:RQ])
                    nc.sync.dma_start(
                        pack_loc[qt][mc * 128 * RQ:(mc + 1) * 128 * RQ]
                        .rearrange("(p r) -> p r", p=128), kq[:])
                vrow(2 * qt + 1, qt, 1)
                fire(qt)

            def qside(hf):
                RH = R // 2
                r0 = hf * RH
                for jj in range(KC):
                    ps = mm_ps.tile([128, 512], f32, tag="mm")
                    for j in range(KC):
                        nc.tensor.matmul(
                            ps[:],
                            wq_sb[:, j, jj * 128:(jj + 1) * 128],
                            h1t_sb[:, j, r0:r0 + RH],
                            start=(j == 0), stop=(j == KC - 1))
                    nc.vector.tensor_scalar(
                        out=qt_sb[:, jj, r0:r0 + RH], in0=ps[:],
                        scalar1=bq_sb[:, jj:jj + 1], scalar2=None, op0=ADD)
                for mc in range(KC):
                    ps = mm_ps.tile([128, 512], f32, tag="mm")
                    for j in range(KC):
                        nc.tensor.matmul(
                            ps[:],
                            rft_sb[:, j, mc * 128:(mc + 1) * 128],
                            qt_sb[:, j, r0:r0 + RH],
                            start=(j == 0), stop=(j == KC - 1))
                    nc.vector.tensor_copy(qft_sb[:, mc, r0:r0 + RH], ps[:])

            # GCN row blocks; kv quarter fired every 2 blocks
            tiles = at_load(0, split=True)
            nxt = at_load(1)
            ps = a_h(0, tiles); post_s0(0, ps)
            tiles, nxt = nxt, at_load(2)
            ps = a_h(1, tiles); post_s0(1, ps); post_s1(0)
            tiles, nxt = nxt, at_load(3)
            ps = a_h(2, tiles); post_s0(2, ps); post_s1(1); post_s2(0)
            vrow(0, 0, 0); post_s2(1); kv_quarter(0)
            tiles, nxt = nxt, at_load(4)
            ps = a_h(3, tiles); post_s0(3, ps); post_s1(2)
            tiles, nxt = nxt, at_load(5)
            ps = a_h(4, tiles); post_s0(4, ps); post_s1(3); post_s2(2)
            vrow(2, 1, 0); post_s2(3); kv_quarter(1)
            tiles, nxt = nxt, at_load(6)
            ps = a_h(5, tiles); post_s0(5, ps); post_s1(4)
            tiles, nxt = nxt, at_load(7)
            ps = a_h(6, tiles); post_s0(6, ps); post_s1(5); post_s2(4)
            vrow(4, 2, 0); post_s2(5); kv_quarter(2)
            tiles = nxt
            ps = a_h(7, tiles); post_s0(7, ps); post_s1(6); post_s2(6)
            vrow(6, 3, 0); post_s1(7); post_s2(7); kv_quarter(3)

            unpack(0)
            p2w = load_p2_weights()
            qside(0)
            qside(1)
            wo_sb, w1_sb, w2_sb = p2w["wo"], p2w["w1"], p2w["w2"]
            bor_r, b1c_sb, b2r_r = p2w["bor"], p2w["b1c"], p2w["b2r"]

        gcn_p.__exit__(None, None, None)

        # ============ Phase 2: attention + FFN ============
        with ExitStack() as p3:
            slabs = p3.enter_context(tc.tile_pool(name="slabs", bufs=2))
            pva_p = p3.enter_context(tc.tile_pool(name="pva", bufs=1))
            sc3 = p3.enter_context(tc.tile_pool(name="sc3", bufs=2))
            sc4 = p3.enter_context(tc.tile_pool(name="sc4", bufs=3))
            st_ps = p3.enter_context(tc.tile_pool(name="st_ps", bufs=2, space="PSUM"))
            num_ps = p3.enter_context(tc.tile_pool(name="num_ps", bufs=1, space="PSUM"))
            tp2_ps = p3.enter_context(tc.tile_pool(name="tp2_ps", bufs=1, space="PSUM"))
            acc_ps = p3.enter_context(tc.tile_pool(name="acc_ps", bufs=2, space="PSUM"))

            RC = 512  # rows per score slab (2 slabs cover R=1024)
            NSL = R // RC

            slab = [slabs.tile([128, NCH, RC], bf16, tag="slab",
                               name=f"slab{i}")
                    for i in range(NSL)]
            pva_sb = pva_p.tile([128, RB, VA], bf16, tag="pva")

            def scores_q(rc, qt):
                """Scores + exp for one quarter's 16 chunks, exp batched
                2 chunks per ACTIVATE."""
                for c in range(NCORES):
                    cg0 = c * 8 + 2 * qt
                    ps = st_ps.tile([128, 2, RC], f32, tag="st")
                    for t in range(2):
                        for j in range(KC):
                            nc.tensor.matmul(
                                ps[:, t, :],
                                kfaq[qt][:, j, (2 * c + t) * 128:
                                         (2 * c + t + 1) * 128],
                                qft_sb[:, j, rc * RC:(rc + 1) * RC],
                                start=(j == 0), stop=(j == KC - 1))
                    nc.scalar.activation(slab[rc][:, cg0:cg0 + 2, :], ps[:],
                                         Exp, scale=1.0 / 16.0)

            def pv_half(rc, hb, hf):
                """P@V accumulation for one half (quarters 2h, 2h+1)."""
                nps = num_ps.tile([128, VA], f32, tag="num")
                n = 0
                for qt in (2 * hf, 2 * hf + 1):
                    for c in range(NCORES):
                        for t in range(2):
                            cg = c * 8 + 2 * qt + t
                            nc.tensor.matmul(
                                nps[:],
                                slab[rc][:, cg, hb * 128:(hb + 1) * 128],
                                vaugq[qt][:, 2 * c + t, :],
                                start=(n == 0), stop=(n == 31))
                            n += 1
                return nps

            def pv_a(rc):
                for hb in range(RC // 128):
                    rb = rc * (RC // 128) + hb
                    nps = pv_half(rc, hb, 0)
                    nc.vector.tensor_copy(pva_sb[:, rb, :], nps[:])

            def attn_stage(rc, hb):
                rb = rc * (RC // 128) + hb
                nps = pv_half(rc, hb, 1)
                xf = sc3.tile([128, VA], f32, tag="xf")
                nc.vector.tensor_add(xf[:], nps[:], pva_sb[:, rb, :])
                rec = sc3.tile([128, 1], f32, tag="rec")
                nc.vector.reciprocal(rec[:], xf[:, D:D + 1])
                attn_bf = sc3.tile([128, D], bf16, tag="attn")
                nc.vector.tensor_scalar(out=attn_bf[:], in0=xf[:, 0:D],
                                        scalar1=rec[:, 0:1], scalar2=None,
                                        op0=mybir.AluOpType.mult)
                attnT = sc3.tile([128, KC, 128], bf16, tag="attnT")
                for j in range(KC):
                    tp = tp2_ps.tile([128, 128], bf16, tag="tp2")
                    nc.tensor.transpose(
                        tp[:], attn_bf[:, j * 128:(j + 1) * 128], ident_bf[:])
                    nc.vector.tensor_copy(attnT[:, j, :], tp[:])
                hg = acc_ps.tile([128, D], f32, tag="acc")
                for j in range(KC):
                    nc.tensor.matmul(hg[:], attnT[:, j, :], wo_sb[:, j, :],
                                     start=(j == 0), stop=False)
                nc.tensor.matmul(hg[:], ones_k1[:], bor_r[:],
                                 start=False, stop=True)
                x2 = sc3.tile([128, D], f32, tag="x2")
                nc.vector.tensor_add(x2[:], hg[:], h1_sb[:, rb, :])
                h2n = sc4.tile([128, D], f32, tag="h2n")
                _ln_block(nc, sc3, x2[:], h2n[:], eps_t=eps_t[:])
                h2 = sc4.tile([128, D], f32, tag="h2")
                nc.gpsimd.tensor_mul(h2[:], h2n[:], gb_sb["g2"][:])
                nc.gpsimd.tensor_add(h2[:], h2[:], gb_sb["be2"][:])
                h2bf = sc4.tile([128, D], bf16, tag="h2bf")
                nc.vector.tensor_copy(h2bf[:], h2n[:])
                return rb, h2, h2bf

            def ffn_stage(st):
                rb, h2, h2bf = st
                h2T = sc3.tile([128, KC, 128], bf16, tag="h2T")
                for j in range(KC):
                    tp = tp2_ps.tile([128, 128], bf16, tag="tp2")
                    nc.tensor.transpose(
                        tp[:], h2bf[:, j * 128:(j + 1) * 128], ident_bf[:])
                    nc.vector.tensor_copy(h2T[:, j, :], tp[:])
                uT = sc3.tile([128, FC, 128], bf16, tag="uT")
                for jf in range(FC):
                    up = acc_ps.tile([128, D], f32, tag="acc")
                    for j in range(KC):
                        nc.tensor.matmul(
                            up[:, 0:128],
                            w1_sb[:, j, jf * 128:(jf + 1) * 128],
                            h2T[:, j, :],
                            start=(j == 0), stop=(j == KC - 1))
                    nc.vector.tensor_scalar(
                        out=uT[:, jf, :], in0=up[:, 0:128],
                        scalar1=b1c_sb[:, jf:jf + 1], scalar2=0.0,
                        op0=mybir.AluOpType.add,
                        op1=mybir.AluOpType.max)
                o2 = acc_ps.tile([128, D], f32, tag="acc")
                for jf in range(FC):
                    nc.tensor.matmul(o2[:], uT[:, jf, :], w2_sb[:, jf, :],
                                     start=(jf == 0), stop=False)
                nc.tensor.matmul(o2[:], ones_k1[:], b2r_r[:],
                                 start=False, stop=True)
                x3 = sc3.tile([128, D], f32, tag="x3")
                nc.vector.tensor_add(x3[:], o2[:], h2[:])
                o_sb = sc3.tile([128, D], f32, tag="osb")
                _ln_block(nc, sc3, x3[:], o_sb[:],
                          gb_sb["g3"][:], gb_sb["be3"][:], eps_t[:],
                          tail=nc.gpsimd)
                nc.sync.dma_start(out_h[rb * 128:(rb + 1) * 128, :], o_sb[:])

            def pv_b_all():
                # one continuous 2-stage pipeline over all 8 row blocks,
                # with the last score group interleaved at the slab
                # boundary so the pipeline never drains mid-way
                prev = None
                for hb in range(RC // 128):
                    st = attn_stage(0, hb)
                    if prev is not None:
                        ffn_stage(prev)
                    prev = st
                scores_q(1, 3)
                for hb in range(RC // 128):
                    st = attn_stage(1, hb)
                    ffn_stage(prev)
                    prev = st
                ffn_stage(prev)

            # arrival order at quarter granularity; later quarters'
            # unpack triggers are staged between score groups so the sync
            # engine's semaphore waits overlap PE work on earlier quarters
            scores_q(0, 0)
            unpack(1)
            scores_q(1, 0)
            scores_q(0, 1)
            unpack(2)
            scores_q(1, 1)
            pv_a(0)
            pv_a(1)
            scores_q(0, 2)
            unpack(3)
            scores_q(1, 2)
            scores_q(0, 3)
            pv_b_all()


_NC_CACHE = None


def _get_nc():
    global _NC_CACHE
    if _NC_CACHE is None:
        _NC_CACHE = _build()
    return _NC_CACHE


def _host_prep(inputs):
    """Build per-core in_maps from full inputs."""
    h = np.ascontiguousarray(np.asarray(inputs["h"], dtype=np.float32))
    ei = np.asarray(inputs["edge_index"]).astype(np.int64)
    src, dst = ei[0], ei[1]

    deg = np.bincount(dst, minlength=N).astype(np.float32) + 1.0
    dinv = 1.0 / np.sqrt(deg)
    coef = (dinv[src] * dinv[dst]).astype(np.float32)
    A = np.zeros((N, N), np.float32)
    np.add.at(A, (dst, src), coef)
    idx = np.arange(N)
    A[idx, idx] += dinv * dinv

    f32c = lambda k: np.ascontiguousarray(np.asarray(inputs[k], dtype=np.float32))
    bfc = lambda x: np.ascontiguousarray(x.astype(BF))

    w = {k: f32c(k) for k in ("W_gcn", "Wq", "Wk", "Wv", "Wo", "RF",
                              "W1", "W2", "b_gcn", "bq", "bk", "bv", "bo",
                              "b1", "b2", "g1", "be1", "g2", "be2", "g3", "be3")}

    hn = np.ascontiguousarray(
        h.reshape(NCH, 128, D).transpose(1, 0, 2).reshape(128, NCH * D)
        .astype(E4))

    # fold the layernorm affines into the consuming projections
    w1f = w["W1"] * w["g2"].reshape(D, 1)
    b1f = w["b1"] + w["be2"] @ w["W1"]
    g1c = w["g1"].reshape(D, 1)
    wqf = w["Wq"] * g1c
    wkf = w["Wk"] * g1c
    wvf = w["Wv"] * g1c
    bqf = w["bq"] + w["be1"] @ w["Wq"]
    bkf = w["bk"] + w["be1"] @ w["Wk"]
    bvf = w["bv"] + w["be1"] @ w["Wv"]

    common = {
        "hn": hn,
        "wgcn": bfc(w["W_gcn"]), "wq": bfc(wqf), "wk": bfc(wkf),
        "wv": bfc(wvf), "wo": bfc(w["Wo"]), "rft": bfc(w["RF"].T),
        "w1": bfc(w1f), "w2": bfc(w["W2"]),
        "bqc": np.ascontiguousarray(bqf.reshape(D, 1)),
        "bkc": np.ascontiguousarray(bkf.reshape(D, 1)),
        "b1c": np.ascontiguousarray(b1f.reshape(F, 1)),
        "bvr": bfc(bvf.reshape(1, D)),
        "bor": bfc(w["bo"].reshape(1, D)),
        "b2r": bfc(w["b2"].reshape(1, D)),
        "g1": bfc(w["g1"].reshape(1, D)),
        "be1": bfc(w["be1"].reshape(1, D)),
        "g2": bfc(w["g2"].reshape(1, D)),
        "be2": bfc(w["be2"].reshape(1, D)),
        "g3": bfc(w["g3"].reshape(1, D)),
        "be3": bfc(w["be3"].reshape(1, D)),
    }

    in_maps = []
    for c in range(NCORES):
        r0 = c * R
        a_loc = A[r0:r0 + R].reshape(RB, 128, 2, NCH // 2, 128)
        at = np.ascontiguousarray(a_loc.transpose(0, 2, 4, 3, 1).astype(E4))
        hr = (h[r0:r0 + R] + w["b_gcn"]).reshape(RB, 128, D).transpose(
            1, 0, 2).reshape(128, RB * D)
        m = dict(common)
        m["at"] = at
        m["hres"] = np.ascontiguousarray(hr)
        in_maps.append(m)
    return in_maps


def kernel(**inputs):
    nc = _get_nc()
    in_maps = _host_prep(inputs)
    res = run_bass_kernel_spmd(nc, in_maps, core_ids=list(range(NCORES)))
    out = np.concatenate([np.asarray(r["out"]) for r in res.results], axis=0)
    return out.astype(np.float32)
